# revision 4
# baseline (speedup 1.0000x reference)
"""Bahdanau attention kernel for 8x Trainium2 NeuronCores.

Computes, for encoded [32, 2048, 1024] f32, W [1024,1024], B [1024], U [1024]:
    score = tanh(encoded @ W + B)         # [b, s, e]
    v     = score @ U                     # [b, s]
    alpha = softmax(v, axis=-1)           # [b, s]
    out   = sum_s alpha[b,s] encoded[b,s] # [b, d]

Sharding: data-parallel over batch, 4 batches per core, W/B/U replicated.

Per-core algorithm (everything stays on one core, no collectives):
  score pass (bf16):  score^T[e, s_chunk] = sum_dt W[dt,et].T @ X^T[dt, s_chunk]
                      (W stationary tiles, host-pretransposed X^T streamed)
                      tanh fused with +B via ACT per-partition bias;
                      v row = sum_et U[et].T @ tanh_tile    (M=1 matmuls)
  softmax:            w = exp(v) rows (no max-sub: |v| <~ 1), Z via ACT accum;
                      w rows bounced through DRAM into [s%128, s//128] layout
  context pass(f32r): ctx_u[1, d] = sum_st w_tile[st].T @ X_nat[st, d_chunk]
                      out = ctx_u / Z
"""

import os
import sys

for _p in (
    "/root/.axon_site",
    "/root/.axon_site/_ro/trn_rl_repo",
    "/root/.axon_site/_ro/pypackages",
    "/opt/trn_rl_repo",
    "/opt/pypackages",
):
    if os.path.isdir(_p) and _p not in sys.path:
        sys.path.append(_p)

import ml_dtypes
import numpy as np

import concourse.bass as bass
import concourse.mybir as mybir
import concourse.tile as tile
from concourse import bacc
from concourse.bass_utils import run_bass_kernel_spmd

F32 = mybir.dt.float32
F32R = mybir.dt.float32r
BF16 = mybir.dt.bfloat16
AF = mybir.ActivationFunctionType
P = 128

NCORES = 8
FULL_SHAPE = dict(b_per_core=4, s=2048, d=1024)

# test.py may override these (e.g. trace=True) before calling kernel().
RUN_KWARGS = {}
LAST_RESULTS = None


def build_bass(b_per_core, s, d, chunk=512):
    """Build the per-core Bass program. d is both the model dim and the
    score dim (W is square in this problem)."""
    e = d
    nch = s // chunk          # score s-chunks per batch
    dt_n = d // P             # contraction tiles
    et_n = e // P             # score^T partition tiles
    st_n = s // P             # context-pass s tiles
    ncx = (d + 511) // 512    # context output free-dim chunks
    cxw = d // ncx            # context chunk width (<= 512)

    nc = bacc.Bacc(
        trn_type="TRN2", target_bir_lowering=False, debug=False, num_devices=1
    )
    xt = nc.dram_tensor("xt", [b_per_core, d, s], BF16, kind="ExternalInput").ap()
    xn = nc.dram_tensor("xn", [b_per_core, s, d], F32R, kind="ExternalInput").ap()
    wf = nc.dram_tensor("w", [d, e], BF16, kind="ExternalInput").ap()
    bf = nc.dram_tensor("b", [e], F32, kind="ExternalInput").ap()
    uf = nc.dram_tensor("u", [e], BF16, kind="ExternalInput").ap()
    out = nc.dram_tensor("out", [b_per_core, d], F32, kind="ExternalOutput").ap()

    with tile.TileContext(nc) as tc:
        with (
            tc.tile_pool(name="singles", bufs=1) as singles,
            tc.tile_pool(name="xtp", bufs=3) as xtp,
            tc.tile_pool(name="thp", bufs=3) as thp,
            tc.tile_pool(name="wrp", bufs=2) as wrp,
            tc.tile_pool(name="wep", bufs=2) as wep,
            tc.tile_pool(name="xnp", bufs=4) as xnp,
            tc.tile_pool(name="outp", bufs=2) as outp,
            tc.tile_pool(name="smallp", bufs=2) as smallp,
            tc.tile_pool(name="scp", bufs=3, space="PSUM") as scp,
            tc.tile_pool(name="vp", bufs=2, space="PSUM") as vp,
            tc.tile_pool(name="cxp", bufs=1, space="PSUM") as cxp,
            tc.tile_pool(name="drp", bufs=2, space="DRAM") as drp,
        ):
            w_sb = singles.tile([P, dt_n, e], BF16)
            nc.sync.dma_start(w_sb, wf.rearrange("(dt p) e -> p dt e", p=P))
            b_sb = singles.tile([P, et_n], F32)
            nc.sync.dma_start(b_sb, bf.rearrange("(et p) -> p et", p=P))
            u_sb = singles.tile([P, et_n], BF16)
            nc.sync.dma_start(u_sb, uf.rearrange("(et p) -> p et", p=P))
            zparts = singles.tile([1, b_per_core * nch], F32)

            # U-dot matmuls lag one e-tile group behind the score matmuls so
            # the PE never waits on the ACT tanh. Entries:
            # (v_psum, et, chunk_done_cb)
            vdot_q = []

            def flush_one_vdot():
                if not vdot_q:
                    return
                vps, u_slice, th, start, stop, done_cb = vdot_q.pop(0)
                nc.tensor.matmul(vps, lhsT=u_slice, rhs=th, start=start, stop=stop)
                if stop and done_cb is not None:
                    done_cb()

            def flush_all_vdot():
                while vdot_q:
                    flush_one_vdot()

            def emit_score_chunk(b, c, wb):
                """Score matmuls + tanh for s-chunk c of batch b; queues the
                U-dot matmuls; on completion of the chunk's v row emits
                exp + bounce-to-DRAM."""
                xt_t = xtp.tile([P, dt_n, chunk], BF16)
                nc.sync.dma_start(
                    xt_t,
                    xt[b].rearrange("(dt p) s -> p dt s", p=P)[
                        :, :, c * chunk : (c + 1) * chunk
                    ],
                )
                vps = vp.tile([1, chunk], F32)

                def chunk_done(b=b, c=c, vps=vps, wb=wb):
                    wrow = wrp.tile([1, chunk], F32R)
                    nc.scalar.activation(
                        wrow,
                        vps,
                        AF.Exp,
                        accum_out=zparts[:, b * nch + c : b * nch + c + 1],
                    )
                    nc.sync.dma_start(wb[c : c + 1, :], wrow)

                for et in range(et_n):
                    sc = scp.tile([P, chunk], F32)
                    for dt in range(dt_n):
                        nc.tensor.matmul(
                            sc,
                            lhsT=w_sb[:, dt, et * P : (et + 1) * P],
                            rhs=xt_t[:, dt, :],
                            start=(dt == 0),
                            stop=(dt == dt_n - 1),
                        )
                    flush_one_vdot()
                    th = thp.tile([P, chunk], BF16)
                    nc.scalar.activation(th, sc, AF.Tanh, bias=b_sb[:, et : et + 1])
                    vdot_q.append(
                        (
                            vps,
                            u_sb[:, et : et + 1],
                            th,
                            et == 0,
                            et == et_n - 1,
                            chunk_done if et == et_n - 1 else None,
                        )
                    )

            def emit_ctx(b, wb):
                """Context pass for batch b: needs all of batch b's exp rows
                already emitted (they were bounced into wb)."""
                wt = wep.tile([P, st_n], F32R)
                nc.sync.dma_start(
                    wt, wb.rearrange("c (g p) -> p (c g)", p=P)
                )
                ctx_ps = [
                    cxp.tile([1, cxw], F32, tag=f"ctx{i}", name=f"ctx{i}")
                    for i in range(ncx)
                ]
                for st in range(st_n):
                    xn_t = xnp.tile([P, d], F32R)
                    nc.sync.dma_start(xn_t, xn[b, st * P : (st + 1) * P, :])
                    lw = wt[:, st : st + 1]
                    rr = xn_t
                    for i in range(ncx):
                        nc.tensor.matmul(
                            ctx_ps[i],
                            lhsT=lw,
                            rhs=rr[:, i * cxw : (i + 1) * cxw],
                            start=(st == 0),
                            stop=(st == st_n - 1),
                        )
                z1 = smallp.tile([1, 1], F32, tag="z1")
                nc.vector.reduce_sum(
                    z1,
                    zparts[:, b * nch : (b + 1) * nch],
                    axis=mybir.AxisListType.X,
                )
                rz = smallp.tile([1, 1], F32, tag="rz")
                nc.vector.reciprocal(rz, z1)
                orow = outp.tile([1, d], F32)
                for i in range(ncx):
                    nc.vector.tensor_scalar_mul(
                        orow[:, i * cxw : (i + 1) * cxw], ctx_ps[i], scalar1=rz
                    )
                nc.sync.dma_start(out[b : b + 1, :], orow)

            # Main emission: one-chunk lag between a batch's score pass and
            # the previous batch's context pass so the exp/bounce latency
            # hides under score matmuls.
            wbs = {}
            for b in range(b_per_core):
                wbs[b] = drp.tile([nch, chunk], F32R, tag="bounce", name="bounce")
                for c in range(nch):
                    emit_score_chunk(b, c, wbs[b])
                    if b > 0 and c == min(1, nch - 1):
                        emit_ctx(b - 1, wbs[b - 1])
            flush_all_vdot()
            emit_ctx(b_per_core - 1, wbs[b_per_core - 1])

    nc.compile()
    return nc


_CACHE = {}


def _get_nc(shape):
    key = tuple(sorted(shape.items()))
    if key not in _CACHE:
        _CACHE[key] = build_bass(**shape)
    return _CACHE[key]


def kernel(encoded, W, B, U):
    global LAST_RESULTS
    encoded = np.asarray(encoded, dtype=np.float32)
    W = np.asarray(W, dtype=np.float32)
    B = np.asarray(B, dtype=np.float32)
    U = np.asarray(U, dtype=np.float32)

    nb = encoded.shape[0]
    bpc = nb // NCORES
    shape = dict(b_per_core=bpc, s=encoded.shape[1], d=encoded.shape[2])
    nc = _get_nc(shape)

    bf16 = ml_dtypes.bfloat16
    w_b = np.ascontiguousarray(W.astype(bf16))
    b_f = np.ascontiguousarray(B)
    u_b = np.ascontiguousarray(U.astype(bf16))
    in_maps = []
    for i in range(NCORES):
        enc = encoded[i * bpc : (i + 1) * bpc]
        in_maps.append(
            {
                "xt": np.ascontiguousarray(enc.transpose(0, 2, 1)).astype(bf16),
                "xn": np.ascontiguousarray(enc),
                "w": w_b,
                "b": b_f,
                "u": u_b,
            }
        )
    res = run_bass_kernel_spmd(
        nc, in_maps, core_ids=list(range(NCORES)), **RUN_KWARGS
    )
    LAST_RESULTS = res
    return np.concatenate([r["out"] for r in res.results], axis=0)
